# revision 2
# baseline (speedup 1.0000x reference)
"""Trainium2 Bass kernel for nn_AttentionModel (S=2048, B=32, H=1024).

Math: reference computes
    energy[b,s] = (enc[s,b,:] @ We.T + (h @ Wh.T + bias)) @ v  ; out = softmax_s(energy)
Since softmax is shift-invariant and the (h @ Wh.T + bias) @ v term is constant
over s, the output reduces exactly to
    out[b, 0, s] = softmax_s( enc[s,b,:] . u ),   u = v[0] @ We   (We = attn_W[:, H:])
So the kernel is a memory-bound [S*B, H] x [H] matvec + row softmax.

Sharding: data-parallel over batch B across 8 cores (4 batches/core).
Device layout per core: enc [BL, H, S] (h on SBUF partitions, s on free dim),
PE matmul contracts h in chunks of 128 (lhsT = u chunk [128,1], rhs = enc tile
[128,512], fp32r, PSUM-accumulated), softmax on ScalarE/VectorE.
"""

import numpy as np

import concourse.bass as bass
import concourse.tile as tile
from concourse import bacc, mybir
from concourse.bass_utils import run_bass_kernel_spmd

S, B, H = 2048, 32, 1024
NCORES = 8
BL = B // NCORES  # batches per core
MM_N = 512        # matmul moving free dim (fp32 max, 1 PSUM bank)


def build_nc(bl=BL, h=H, s=S, enc_bufs=4, jpd=4, use_fp16=True, debug=False,
             taper=True):
    """Build the per-core Bass program (SPMD: same program, different data)."""
    nc = bacc.Bacc()
    f32 = mybir.dt.float32
    jc = h // 128      # h chunks (contraction tiles)
    ns = s // MM_N     # matmul slices per output row
    jpd = min(jpd, jc) # h-chunks per DMA
    nd = jc // jpd     # DMAs per batch
    # Per-batch DMA chunking (in h-chunks of 128). Large chunks sustain the
    # best HBM rate; the last batch tapers so the cold-PE tail after the
    # final chunk is only a few matmuls.
    # Last batch: small chunks first, then one big 4-h-chunk block streamed as
    # ns per-slice sub-DMAs. Keeps the DMA queue DEEP at the end of the
    # stream (shallow tail queues get starved by the HBM-pair partner under
    # contention) while still letting each slice finish as its bytes land.
    plan = [[jpd] * nd for _ in range(bl)]
    split_last = taper and jc == 8 and jpd in (4, 8)
    if split_last:
        plan[bl - 1] = [1, 1, 2, 4]

    # enc streams in fp16 (host casts; softmax tolerance has 8x margin) —
    # halves HBM traffic, which is the roofline for this kernel.
    mm_dt = mybir.dt.float16 if use_fp16 else mybir.dt.float32r
    enc_d = nc.declare_dram_parameter("enc", [bl, h, s], mm_dt, isOutput=False)
    u_d = nc.declare_dram_parameter("u", [128, jc], mm_dt, isOutput=False)
    # Flash-softmax split: out rows are exp(e_slice - max_slice) per 512-wide
    # slice; per-slice sums and (negated) maxes are returned and the host
    # does the 4-element rescale + divide during the gather (the heavy work -
    # energy matvec, maxes, exp, sums - all happens on device).
    out_d = nc.declare_dram_parameter("out", [bl, s], f32, isOutput=True)
    sums_d = nc.declare_dram_parameter("sums", [bl, ns], f32, isOutput=True)
    maxs_d = nc.declare_dram_parameter("maxs", [bl, ns], f32, isOutput=True)
    if debug:
        dbg_e = nc.declare_dram_parameter("dbg_e", [bl, s], f32, isOutput=True)
        dbg_p = nc.declare_dram_parameter("dbg_p", [bl, s], f32, isOutput=True)
        dbg_m = nc.declare_dram_parameter("dbg_m", [bl, 4], f32, isOutput=True)

    # Bacc's compile() legalizes multi-semaphore waits (splitting them into
    # EventSemaphore chains), so the structure below can stay simple. Big enc
    # loads go via HWDGE (nc.sync); small transfers via SWDGE (nc.gpsimd).
    with tile.TileContext(nc) as tc:
        with (
            tc.tile_pool(name="up", bufs=1) as up,
            tc.tile_pool(name="encp", bufs=enc_bufs) as encp,
            tc.tile_pool(name="smp", bufs=2) as smp,
            tc.tile_pool(name="op", bufs=1) as op,
            tc.tile_pool(name="psp", bufs=2, space="PSUM") as psp,
        ):
            # Issue the first enc load before anything else so the DMA
            # pipeline starts immediately; the tiny u load follows it.
            t0 = encp.tile([128, plan[0][0], s], mm_dt, name="t",
                           padded_shape=[128, jpd, s])
            nc.sync.dma_start(
                t0[:],
                enc_d[0, 0:plan[0][0] * 128, :].rearrange("(j p) s -> p j s", p=128),
            )
            u_sb = up.tile([128, jc], mm_dt)
            nc.sync.dma_start(u_sb[:], u_d[:])

            o_sum = op.tile([1, bl, ns], f32)
            o_max = op.tile([1, bl, ns], f32)
            for b in range(bl):
                # Accumulate this batch's energy row in PSUM [1, s] (4 banks,
                # partition 0); 8 fp32r matmuls per 512-wide slice.
                e_ps = psp.tile([1, s], f32)
                m4 = smp.tile([1, ns], f32)
                s4 = smp.tile([1, ns], f32)
                p_exp = smp.tile([1, s], f32)
                last = b == bl - 1 and split_last
                j = 0
                for d, cw in enumerate(plan[b]):
                    split = ns if (last and d == len(plan[b]) - 1) else 1
                    for sub in range(split):
                        # For the final block, stream each 512-wide s-slice
                        # as its own DMAs - and split off the very last
                        # h-chunk (the slice's finishing matmul) into its own
                        # small DMA, so only ONE matmul + max + exp sit after
                        # the slice's last bytes.
                        if b == 0 and d == 0:
                            t = t0
                        elif split == 1:
                            scols = s
                            t = encp.tile([128, cw, scols], mm_dt, name="t",
                                          padded_shape=[128, jpd, s])
                            src = enc_d[b, j * 128:(j + cw) * 128, :]
                            nc.sync.dma_start(
                                t[:], src.rearrange("(j p) s -> p j s", p=128)
                            )
                        else:
                            scols = s // split
                            t = encp.tile([128, cw, scols], mm_dt, name="t",
                                          padded_shape=[128, jpd, s])
                            sc = slice(sub * scols, (sub + 1) * scols)
                            src_a = enc_d[b, j * 128:(j + cw - 1) * 128, sc]
                            nc.sync.dma_start(
                                t[:, 0:cw - 1, :],
                                src_a.rearrange("(j p) s -> p j s", p=128),
                            )
                            src_b = enc_d[b, (j + cw - 1) * 128:(j + cw) * 128, sc]
                            nc.sync.dma_start(
                                t[:, cw - 1:cw, :],
                                src_b.rearrange("(j p) s -> p j s", p=128),
                            )
                        for jl in range(cw):
                            sss = range(ns) if split == 1 else [sub]
                            for ss in sss:
                                coff = 0 if split == 1 else -ss * MM_N
                                nc.tensor.matmul(
                                    e_ps[:, ss * MM_N:(ss + 1) * MM_N],
                                    u_sb[:, j + jl:j + jl + 1],
                                    t[:, jl, ss * MM_N + coff:
                                       (ss + 1) * MM_N + coff],
                                    start=(j + jl == 0),
                                    stop=(j + jl == jc - 1),
                                )
                                if j + jl == jc - 1:
                                    # This slice's group is complete: negated
                                    # max, then exp with fused slice-sum,
                                    # overlapping remaining matmuls/DMAs.
                                    nc.vector.reduce_max(
                                        m4[:, ss:ss + 1],
                                        e_ps[:, ss * MM_N:(ss + 1) * MM_N],
                                        axis=mybir.AxisListType.X,
                                        negate=True,
                                    )
                                    nc.scalar.activation(
                                        p_exp[:, ss * MM_N:(ss + 1) * MM_N],
                                        e_ps[:, ss * MM_N:(ss + 1) * MM_N],
                                        mybir.ActivationFunctionType.Exp,
                                        bias=m4[:, ss:ss + 1],
                                        accum_out=s4[:, ss:ss + 1],
                                    )
                    j += cw
                nc.gpsimd.dma_start(out_d[b:b + 1, :], p_exp[:])
                nc.vector.tensor_copy(o_sum[:, b, :], s4[:])
                nc.vector.tensor_copy(o_max[:, b, :], m4[:])
            # Keep the partition dim explicit on the SBUF side: x[0] would
            # make the free dim `bl` look like a partition dim.
            nc.gpsimd.dma_start(sums_d[:], o_sum[0:1, :, :])
            nc.gpsimd.dma_start(maxs_d[:], o_max[0:1, :, :])
    nc.compile()
    return nc


def _prep_inputs(encoder_outputs, attn_W, v):
    encoder_outputs = np.asarray(encoder_outputs, dtype=np.float32)
    attn_W = np.asarray(attn_W, dtype=np.float32)
    v = np.asarray(v, dtype=np.float32)
    h = attn_W.shape[0]
    # u = v[0] @ We in float64 (host-side, tiny)
    u = (v[0].astype(np.float64) @ attn_W[:, h:].astype(np.float64)).astype(np.float32)
    u128 = np.ascontiguousarray(u.reshape(h // 128, 128).T)  # [128, jc]
    in_maps = []
    for c in range(NCORES):
        sl = encoder_outputs[:, c * BL:(c + 1) * BL, :]
        enc_c = np.ascontiguousarray(sl.transpose(1, 2, 0))  # [BL, H, S]
        in_maps.append({"enc": enc_c, "u": u128})
    return in_maps


def run(encoder_outputs, rnn_hidden, attn_W, attn_b, v, trace=False, **bass_kwargs):
    in_maps = _prep_inputs(encoder_outputs, attn_W, v)
    nc = build_nc()
    res = run_bass_kernel_spmd(
        nc, in_maps, list(range(NCORES)), trace=trace, **bass_kwargs
    )
    num = np.concatenate([r["out"] for r in res.results], axis=0)    # [B, S]
    sums = np.concatenate([r["sums"] for r in res.results], axis=0)  # [B, ns]
    negm = np.concatenate([r["maxs"] for r in res.results], axis=0)  # [B, ns]
    # flash-softmax combine of the per-512-slice partials
    nb, nsl = sums.shape
    nm = -negm.astype(np.float64)
    m = nm.max(axis=1, keepdims=True)
    scale = np.exp(nm - m)                                  # [B, ns]
    num3 = num.reshape(nb, nsl, -1) * scale[:, :, None]
    tot = (sums.astype(np.float64) * scale).sum(axis=1)     # [B]
    out = num3.reshape(nb, -1) / tot[:, None]
    return out[:, None, :].astype(np.float32), res


def kernel(encoder_outputs, rnn_hidden, attn_W, attn_b, v):
    out, _ = run(encoder_outputs, rnn_hidden, attn_W, attn_b, v)
    return out



# revision 3
# speedup vs baseline: 1.4091x; 1.4091x over previous
"""Trainium2 Bass kernel for nn_AttentionModel (S=2048, B=32, H=1024).

Math: reference computes
    energy[b,s] = (enc[s,b,:] @ We.T + (h @ Wh.T + bias)) @ v  ; out = softmax_s(energy)
Since softmax is shift-invariant and the (h @ Wh.T + bias) @ v term is constant
over s, the output reduces exactly to
    out[b, 0, s] = softmax_s( enc[s,b,:] . u ),   u = v[0] @ We   (We = attn_W[:, H:])
So the kernel is a memory-bound [S*B, H] x [H] matvec + row softmax.

Sharding: data-parallel over batch B across 8 cores (4 batches/core).
Device layout per core: enc [BL, H, S] (h on SBUF partitions, s on free dim),
PE matmul contracts h in chunks of 128 (lhsT = u chunk [128,1], rhs = enc tile
[128,512], fp32r, PSUM-accumulated), softmax on ScalarE/VectorE.
"""

import numpy as np

import concourse.bass as bass
import concourse.tile as tile
from concourse import bacc, mybir
from concourse.bass_utils import run_bass_kernel_spmd

S, B, H = 2048, 32, 1024
NCORES = 8
BL = B // NCORES  # batches per core
MM_N = 512        # matmul moving free dim (fp32 max, 1 PSUM bank)


def build_nc(bl=BL, h=H, s=S, enc_bufs=4, jpd=4, use_fp16=True, debug=False,
             taper=True):
    """Build the per-core Bass program (SPMD: same program, different data)."""
    nc = bacc.Bacc()
    f32 = mybir.dt.float32
    jc = h // 128      # h chunks (contraction tiles)
    ns = s // MM_N     # matmul slices per output row
    jpd = min(jpd, jc) # h-chunks per DMA
    nd = jc // jpd     # DMAs per batch
    # Per-batch DMA chunking (in h-chunks of 128). Large chunks sustain the
    # best HBM rate; the last batch tapers so the cold-PE tail after the
    # final chunk is only a few matmuls.
    # Last batch: small chunks first, then one big 4-h-chunk block streamed as
    # ns per-slice sub-DMAs. Keeps the DMA queue DEEP at the end of the
    # stream (shallow tail queues get starved by the HBM-pair partner under
    # contention) while still letting each slice finish as its bytes land.
    plan = [[jpd] * nd for _ in range(bl)]
    split_last = taper and jc == 8 and jpd in (4, 8)
    if split_last:
        plan[bl - 1] = [1, 1, 2, 4]

    # enc streams in fp16 (host casts; softmax tolerance has 8x margin) —
    # halves HBM traffic, which is the roofline for this kernel.
    mm_dt = mybir.dt.float16 if use_fp16 else mybir.dt.float32r
    enc_d = nc.declare_dram_parameter("enc", [bl, h, s], mm_dt, isOutput=False)
    u_d = nc.declare_dram_parameter("u", [128, jc], mm_dt, isOutput=False)
    # Flash-softmax split: out rows are exp(e_slice - max_slice) per 512-wide
    # slice; per-slice sums and (negated) maxes are returned and the host
    # does the 4-element rescale + divide during the gather (the heavy work -
    # energy matvec, maxes, exp, sums - all happens on device).
    out_d = nc.declare_dram_parameter("out", [bl, s], f32, isOutput=True)
    sums_d = nc.declare_dram_parameter("sums", [bl, ns], f32, isOutput=True)
    maxs_d = nc.declare_dram_parameter("maxs", [bl, ns], f32, isOutput=True)
    if debug:
        dbg_e = nc.declare_dram_parameter("dbg_e", [bl, s], f32, isOutput=True)
        dbg_p = nc.declare_dram_parameter("dbg_p", [bl, s], f32, isOutput=True)
        dbg_m = nc.declare_dram_parameter("dbg_m", [bl, 4], f32, isOutput=True)

    # Bacc's compile() legalizes multi-semaphore waits (splitting them into
    # EventSemaphore chains), so the structure below can stay simple. Big enc
    # loads go via HWDGE (nc.sync); small transfers via SWDGE (nc.gpsimd).
    with tile.TileContext(nc) as tc:
        with (
            tc.tile_pool(name="up", bufs=1) as up,
            tc.tile_pool(name="encp", bufs=enc_bufs) as encp,
            tc.tile_pool(name="smp", bufs=2) as smp,
            tc.tile_pool(name="op", bufs=1) as op,
            tc.tile_pool(name="psp", bufs=2, space="PSUM") as psp,
        ):
            # Issue the first enc load before anything else so the DMA
            # pipeline starts immediately; the tiny u load follows it.
            t0 = encp.tile([128, plan[0][0], s], mm_dt, name="t",
                           padded_shape=[128, jpd, s])
            nc.sync.dma_start(
                t0[:],
                enc_d[0, 0:plan[0][0] * 128, :].rearrange("(j p) s -> p j s", p=128),
            )
            u_sb = up.tile([128, jc], mm_dt)
            nc.sync.dma_start(u_sb[:], u_d[:])

            o_sum = op.tile([1, bl, ns], f32)
            o_max = op.tile([1, bl, ns], f32)
            for b in range(bl):
                # Accumulate this batch's energy row in PSUM [1, s] (4 banks,
                # partition 0); 8 fp32r matmuls per 512-wide slice.
                e_ps = psp.tile([1, s], f32)
                m4 = smp.tile([1, ns], f32)
                s4 = smp.tile([1, ns], f32)
                p_exp = smp.tile([1, s], f32)
                last = b == bl - 1 and split_last
                j = 0
                for d, cw in enumerate(plan[b]):
                    split = ns if (last and d == len(plan[b]) - 1) else 1
                    for sub in range(split):
                        # For the final block, stream each 512-wide s-slice
                        # as its own DMAs - and split off the very last
                        # h-chunk (the slice's finishing matmul) into its own
                        # small DMA, so only ONE matmul + max + exp sit after
                        # the slice's last bytes.
                        if b == 0 and d == 0:
                            t = t0
                        elif split == 1:
                            scols = s
                            t = encp.tile([128, cw, scols], mm_dt, name="t",
                                          padded_shape=[128, jpd, s])
                            src = enc_d[b, j * 128:(j + cw) * 128, :]
                            nc.sync.dma_start(
                                t[:], src.rearrange("(j p) s -> p j s", p=128)
                            )
                        else:
                            scols = s // split
                            t = encp.tile([128, cw, scols], mm_dt, name="t",
                                          padded_shape=[128, jpd, s])
                            sc = slice(sub * scols, (sub + 1) * scols)
                            src_a = enc_d[b, j * 128:(j + cw - 1) * 128, sc]
                            nc.sync.dma_start(
                                t[:, 0:cw - 1, :],
                                src_a.rearrange("(j p) s -> p j s", p=128),
                            )
                            src_b = enc_d[b, (j + cw - 1) * 128:(j + cw) * 128, sc]
                            nc.sync.dma_start(
                                t[:, cw - 1:cw, :],
                                src_b.rearrange("(j p) s -> p j s", p=128),
                            )
                        for jl in range(cw):
                            sss = range(ns) if split == 1 else [sub]
                            for ss in sss:
                                coff = 0 if split == 1 else -ss * MM_N
                                nc.tensor.matmul(
                                    e_ps[:, ss * MM_N:(ss + 1) * MM_N],
                                    u_sb[:, j + jl:j + jl + 1],
                                    t[:, jl, ss * MM_N + coff:
                                       (ss + 1) * MM_N + coff],
                                    start=(j + jl == 0),
                                    stop=(j + jl == jc - 1),
                                )
                                if j + jl == jc - 1:
                                    # This slice's group is complete: negated
                                    # max, then exp with fused slice-sum,
                                    # overlapping remaining matmuls/DMAs.
                                    nc.vector.reduce_max(
                                        m4[:, ss:ss + 1],
                                        e_ps[:, ss * MM_N:(ss + 1) * MM_N],
                                        axis=mybir.AxisListType.X,
                                        negate=True,
                                    )
                                    nc.scalar.activation(
                                        p_exp[:, ss * MM_N:(ss + 1) * MM_N],
                                        e_ps[:, ss * MM_N:(ss + 1) * MM_N],
                                        mybir.ActivationFunctionType.Exp,
                                        bias=m4[:, ss:ss + 1],
                                        accum_out=s4[:, ss:ss + 1],
                                    )
                    j += cw
                nc.gpsimd.dma_start(out_d[b:b + 1, :], p_exp[:])
                nc.vector.tensor_copy(o_sum[:, b, :], s4[:])
                nc.vector.tensor_copy(o_max[:, b, :], m4[:])
            # Keep the partition dim explicit on the SBUF side: x[0] would
            # make the free dim `bl` look like a partition dim.
            nc.gpsimd.dma_start(sums_d[:], o_sum[0:1, :, :])
            nc.gpsimd.dma_start(maxs_d[:], o_max[0:1, :, :])
    nc.compile()
    return nc


def _prep_inputs(encoder_outputs, attn_W, v, use_fp16=True):
    encoder_outputs = np.asarray(encoder_outputs, dtype=np.float32)
    attn_W = np.asarray(attn_W, dtype=np.float32)
    v = np.asarray(v, dtype=np.float32)
    h = attn_W.shape[0]
    dt = np.float16 if use_fp16 else np.float32
    # u = v[0] @ We in float64 (host-side, tiny)
    u = (v[0].astype(np.float64) @ attn_W[:, h:].astype(np.float64)).astype(dt)
    u128 = np.ascontiguousarray(u.reshape(h // 128, 128).T)  # [128, jc]
    in_maps = []
    for c in range(NCORES):
        sl = encoder_outputs[:, c * BL:(c + 1) * BL, :]
        enc_c = np.ascontiguousarray(sl.transpose(1, 2, 0).astype(dt))  # [BL, H, S]
        in_maps.append({"enc": enc_c, "u": u128})
    return in_maps


def run(encoder_outputs, rnn_hidden, attn_W, attn_b, v, trace=False, **bass_kwargs):
    in_maps = _prep_inputs(encoder_outputs, attn_W, v)
    nc = build_nc()
    res = run_bass_kernel_spmd(
        nc, in_maps, list(range(NCORES)), trace=trace, **bass_kwargs
    )
    num = np.concatenate([r["out"] for r in res.results], axis=0)    # [B, S]
    sums = np.concatenate([r["sums"] for r in res.results], axis=0)  # [B, ns]
    negm = np.concatenate([r["maxs"] for r in res.results], axis=0)  # [B, ns]
    # flash-softmax combine of the per-512-slice partials
    nb, nsl = sums.shape
    nm = -negm.astype(np.float64)
    m = nm.max(axis=1, keepdims=True)
    scale = np.exp(nm - m)                                  # [B, ns]
    num3 = num.reshape(nb, nsl, -1) * scale[:, :, None]
    tot = (sums.astype(np.float64) * scale).sum(axis=1)     # [B]
    out = num3.reshape(nb, -1) / tot[:, None]
    return out[:, None, :].astype(np.float32), res


def kernel(encoder_outputs, rnn_hidden, attn_W, attn_b, v):
    out, _ = run(encoder_outputs, rnn_hidden, attn_W, attn_b, v)
    return out

